# revision 1
# baseline (speedup 1.0000x reference)
"""Trainium2 Bass kernel for nn_DeepKMeans (vq_codebook).

Computation (per reference):
    embedding      = x @ W_enc + b_enc                  [B, E]
    reconstruction = embedding @ W_dec + b_dec          [B, D]
    distances[b,k] = sum_e (emb[b,e] - reps[k,e])^2     [B, K]
                   = ||emb_b||^2 + ||reps_k||^2 - 2 emb_b . reps_k
    exps           = exp(-1000 * (distances - rowmin))
    weighted       = distances * exps / rowsum(exps)
Returns (weighted, distances, reconstruction).

Sharding: data-parallel over batch across 8 NeuronCores (128 rows each);
weights / cluster reps replicated. No collectives.

Per-core dataflow (all matmuls fp32, PSUM accumulate):
  enc:   emb[128,512]  = sum_{d_c<32} xT[d_c]^T @ Wenc[d_c]   (xT pre-transposed on host)
  e2:    ACT Square with accum_out -> ||emb_b||^2 [128,1]
  trans: 4x PE transpose -> embT [512x128 as 4 chunks]
  er2:   sum_{e_c<4} embT[e_c]^T @ (-2*reps^T)[e_c] -> [128(b),512(k)]
  dist:  dists = er2 + e2 + r2_bcast   (one DVE scalar_tensor_tensor)
  exp:   ACT Exp(scale=-1000, bias=1000*rowmin) with accum_out=rowsum
  wout:  (exps * 1/rowsum) * dists     (one DVE scalar_tensor_tensor)
  rec:   8 slices of 512: sum_{e_c<4} embT[e_c]^T @ Wdec[e_c, slice]
"""

import numpy as np

B, D, E, K = 1024, 4096, 512, 512
N_CORES = 8
BC = B // N_CORES          # 128 batch rows per core
ALPHA = 1000.0

_cache = {}


def _build(with_bdec: bool):
    import concourse.bacc as bacc
    import concourse.mybir as mybir
    import concourse.tile as tile

    dt = mybir.dt.float32
    Alu = mybir.AluOpType
    Act = mybir.ActivationFunctionType

    nc = bacc.Bacc("TRN2", target_bir_lowering=False, debug=False,
                   enable_asserts=False)

    # ---- DRAM I/O (per-core shapes; host pre-tiled layouts) ----
    # xT:    [p = d%128, d_c*128 + b]          (x shard transposed)
    # wenc:  [p = d%128, d_c*512 + e]
    # wdec:  [p = e%128, s*2048 + e_c*512 + n]
    # repsTs:[p = e%128, e_c*512 + k]          (= -2 * reps^T, tiled)
    xT_d = nc.dram_tensor("xT", [128, 32 * 128], dt, kind="ExternalInput")
    wenc_d = nc.dram_tensor("wenc", [128, 32 * 512], dt, kind="ExternalInput")
    wdec_d = nc.dram_tensor("wdec", [128, 8 * 4 * 512], dt, kind="ExternalInput")
    reps_d = nc.dram_tensor("repsTs", [128, 4 * 512], dt, kind="ExternalInput")
    r2b_d = nc.dram_tensor("r2b", [128, K], dt, kind="ExternalInput")
    benc_d = nc.dram_tensor("bencb", [128, E], dt, kind="ExternalInput")
    ident_d = nc.dram_tensor("ident", [128, 128], dt, kind="ExternalInput")
    if with_bdec:
        bdec_d = nc.dram_tensor("bdecb", [128, D], dt, kind="ExternalInput")
    out_w_d = nc.dram_tensor("out_w", [BC, K], dt, kind="ExternalOutput")
    out_d_d = nc.dram_tensor("out_d", [BC, K], dt, kind="ExternalOutput")
    out_r_d = nc.dram_tensor("out_r", [BC, D], dt, kind="ExternalOutput")

    with tile.TileContext(nc) as tc:
        with (
            tc.tile_pool(name="const", bufs=1) as cpool,
            tc.tile_pool(name="work", bufs=1) as wpool,
            tc.tile_pool(name="rec", bufs=3) as rpool,
            tc.tile_pool(name="ps_emb", bufs=1, space="PSUM") as ps_emb,
            tc.tile_pool(name="ps_tr", bufs=2, space="PSUM") as ps_tr,
            tc.tile_pool(name="ps_er2", bufs=1, space="PSUM") as ps_er2,
            tc.tile_pool(name="ps_rec", bufs=2, space="PSUM") as ps_rec,
        ):
            # ---- resident inputs; DMAs ordered by first use ----
            xT = cpool.tile([128, 32 * 128], dt)
            wenc = cpool.tile([128, 32 * 512], dt)
            wdec = cpool.tile([128, 8 * 4 * 512], dt)
            repsTs = cpool.tile([128, 4 * 512], dt)
            r2b = cpool.tile([128, K], dt)
            bencb = cpool.tile([128, E], dt)
            ident = cpool.tile([128, 128], dt)
            if with_bdec:
                bdecb = cpool.tile([128, D], dt)

            nc.sync.dma_start(xT[:, 0:1024], xT_d.ap()[:, 0:1024])
            nc.sync.dma_start(wenc[:, 0:2048], wenc_d.ap()[:, 0:2048])
            for i in range(1, 4):
                nc.sync.dma_start(xT[:, i * 1024:(i + 1) * 1024],
                                  xT_d.ap()[:, i * 1024:(i + 1) * 1024])
            for i in range(1, 8):
                nc.sync.dma_start(wenc[:, i * 2048:(i + 1) * 2048],
                                  wenc_d.ap()[:, i * 2048:(i + 1) * 2048])
            nc.sync.dma_start(ident[:], ident_d.ap())
            nc.sync.dma_start(bencb[:], benc_d.ap())
            nc.sync.dma_start(repsTs[:], reps_d.ap())
            nc.sync.dma_start(r2b[:], r2b_d.ap())
            if with_bdec:
                for i in range(4):
                    nc.sync.dma_start(bdecb[:, i * 1024:(i + 1) * 1024],
                                      bdec_d.ap()[:, i * 1024:(i + 1) * 1024])
            for i in range(8):
                nc.sync.dma_start(wdec[:, i * 2048:(i + 1) * 2048],
                                  wdec_d.ap()[:, i * 2048:(i + 1) * 2048])

            # ---- encoder: emb = x @ W_enc (+ b_enc) ----
            ps_e = ps_emb.tile([128, E], dt)
            for d_c in range(32):
                nc.tensor.matmul(ps_e[:],
                                 xT[:, d_c * 128:(d_c + 1) * 128],
                                 wenc[:, d_c * 512:(d_c + 1) * 512],
                                 start=(d_c == 0), stop=(d_c == 31))
            emb = wpool.tile([128, E], dt)
            nc.vector.scalar_tensor_tensor(emb[:], ps_e[:], 0.0, bencb[:],
                                           op0=Alu.bypass, op1=Alu.add)

            # ---- ||emb_b||^2 via ACT Square with free-dim accumulate ----
            scal = wpool.tile([128, 8], dt)   # packed [128,1] scalars
            e2 = scal[:, 0:1]
            mmin = scal[:, 1:2]
            biasf = scal[:, 2:3]
            ssum = scal[:, 3:4]
            recip = scal[:, 4:5]
            sqs = wpool.tile([128, E], dt)
            nc.scalar.activation(sqs[:], emb[:], Act.Square, accum_out=e2)

            # ---- transpose emb -> embT (4x 128x128 PE transposes) ----
            embT = wpool.tile([128, E], dt)   # [p=e%128, e_c*128 + b]
            for e_c in range(4):
                pt = ps_tr.tile([128, 128], dt)
                nc.tensor.transpose(pt[:], emb[:, e_c * 128:(e_c + 1) * 128],
                                    ident[:])
                nc.scalar.copy(embT[:, e_c * 128:(e_c + 1) * 128], pt[:])

            # ---- distances ----
            er2 = ps_er2.tile([128, K], dt)
            for e_c in range(4):
                nc.tensor.matmul(er2[:],
                                 embT[:, e_c * 128:(e_c + 1) * 128],
                                 repsTs[:, e_c * 512:(e_c + 1) * 512],
                                 start=(e_c == 0), stop=(e_c == 3))
            dists = wpool.tile([128, K], dt)
            nc.vector.scalar_tensor_tensor(dists[:], er2[:], e2, r2b[:],
                                           op0=Alu.add, op1=Alu.add)
            nc.vector.tensor_reduce(mmin, dists[:], axis=mybir.AxisListType.X,
                                    op=Alu.min)
            nc.scalar.mul(biasf, mmin, ALPHA)
            exps = wpool.tile([128, K], dt)
            nc.scalar.activation(exps[:], dists[:], Act.Exp,
                                 bias=biasf, scale=-ALPHA, accum_out=ssum)
            nc.vector.reciprocal(recip, ssum)
            wout = wpool.tile([128, K], dt)
            nc.vector.scalar_tensor_tensor(wout[:], exps[:], recip, dists[:],
                                           op0=Alu.mult, op1=Alu.mult)
            nc.gpsimd.dma_start(out_d_d.ap(), dists[:])
            nc.gpsimd.dma_start(out_w_d.ap(), wout[:])

            # ---- reconstruction: 8 slices of 512 ----
            for sl in range(8):
                pr = ps_rec.tile([128, 512], dt)
                for e_c in range(4):
                    nc.tensor.matmul(
                        pr[:],
                        embT[:, e_c * 128:(e_c + 1) * 128],
                        wdec[:, sl * 2048 + e_c * 512: sl * 2048 + (e_c + 1) * 512],
                        start=(e_c == 0), stop=(e_c == 3))
                ro = rpool.tile([128, 512], dt)
                if with_bdec:
                    nc.vector.scalar_tensor_tensor(
                        ro[:], pr[:], 0.0, bdecb[:, sl * 512:(sl + 1) * 512],
                        op0=Alu.bypass, op1=Alu.add)
                else:
                    nc.vector.tensor_copy(ro[:], pr[:])
                nc.gpsimd.dma_start(out_r_d.ap()[:, sl * 512:(sl + 1) * 512],
                                    ro[:])

    nc.compile()
    return nc


def _get_nc(with_bdec: bool):
    key = ("nc", with_bdec)
    if key not in _cache:
        _cache[key] = _build(with_bdec)
    return _cache[key]


def _f32c(a):
    return np.ascontiguousarray(np.asarray(a, dtype=np.float32))


def run(x, cluster_reps, W_enc, b_enc, W_dec, b_dec, trace=False):
    from concourse import bass_utils

    x = _f32c(x)
    cluster_reps = _f32c(cluster_reps)
    W_enc = _f32c(W_enc)
    b_enc = _f32c(b_enc)
    W_dec = _f32c(W_dec)
    b_dec = _f32c(b_dec)

    with_bdec = bool(np.any(b_dec))
    nc = _get_nc(with_bdec)

    # host-side retiling (shared across cores)
    wenc_t = _f32c(W_enc.reshape(32, 128, 512).transpose(1, 0, 2)
                   .reshape(128, 32 * 512))
    wdec_t = _f32c(W_dec.reshape(4, 128, 8, 512).transpose(1, 2, 0, 3)
                   .reshape(128, 8 * 4 * 512))
    repsTs_t = _f32c((-2.0 * cluster_reps.T).reshape(4, 128, 512)
                     .transpose(1, 0, 2).reshape(128, 4 * 512))
    r2 = np.sum(cluster_reps.astype(np.float64) ** 2, axis=1).astype(np.float32)
    r2b = _f32c(np.broadcast_to(r2[None, :], (128, K)))
    bencb = _f32c(np.broadcast_to(b_enc[None, :], (128, E)))
    ident = np.eye(128, dtype=np.float32)

    shared = {"wenc": wenc_t, "wdec": wdec_t, "repsTs": repsTs_t,
              "r2b": r2b, "bencb": bencb, "ident": ident}
    if with_bdec:
        shared["bdecb"] = _f32c(np.broadcast_to(b_dec[None, :], (128, D)))

    in_maps = []
    for c in range(N_CORES):
        xs = x[c * BC:(c + 1) * BC]                       # [128, 4096]
        xT_t = _f32c(xs.T.reshape(32, 128, 128).transpose(1, 0, 2)
                     .reshape(128, 32 * 128))
        in_maps.append({**shared, "xT": xT_t})

    res = bass_utils.run_bass_kernel_spmd(
        nc, in_maps, core_ids=list(range(N_CORES)), trace=trace)

    weighted = np.concatenate([res.results[c]["out_w"] for c in range(N_CORES)], 0)
    dists = np.concatenate([res.results[c]["out_d"] for c in range(N_CORES)], 0)
    recon = np.concatenate([res.results[c]["out_r"] for c in range(N_CORES)], 0)
    return (weighted, dists, recon), res


def kernel(x, cluster_reps, W_enc, b_enc, W_dec, b_dec):
    outs, _ = run(x, cluster_reps, W_enc, b_enc, W_dec, b_dec, trace=False)
    return outs
